# revision 3
# baseline (speedup 1.0000x reference)
"""MoRALinear fused kernel for 8x TRN2 NeuronCores.

Math: reference computes
    y = x @ W.T + b + tile(lora_A(chunk_sum(x)))
Both the chunk-sum (x -> [B,S,r] by summing 4 chunks of 1024) and the
output tiling (repeat r->4096) are linear maps made of stacked identity
blocks, so the adapter folds into the base weight exactly:
    y = x @ (W + tile(A, (4,4))).T + b
The device kernel is then a single dense GEMM [16384,4096]x[4096,4096]
plus a bias, data-parallel over tokens across 8 cores (weights
replicated; no collectives).

Per-core kernel (M=2048 tokens, K=4096, N=4096, fp32 via float32r):
  - x block of 1024 tokens cached in SBUF ([128, 32, 1024], 16 MB)
  - W streamed as [128, 4, 512] k-chunks, reused across 8 m-strips
  - 8 PSUM banks hold the 8 m-strips of one n-tile; k accumulates in PSUM
  - bias added during PSUM->SBUF eviction on the vector engine
"""

import numpy as np

import concourse.bass as bass
import concourse.mybir as mybir
import concourse.tile as tile
from concourse import bacc
from concourse.bass_utils import run_bass_kernel_spmd

B, S, IN_F, OUT_F = 4, 4096, 4096, 4096
N_CORES = 8
TOKENS = B * S                  # 16384
M_PER_CORE = TOKENS // N_CORES  # 2048

P = 128
KO = IN_F // P                  # 32 k-strips
NTILE = 512
NT = OUT_F // NTILE             # 8 n-tiles
KO_CHUNK = 4                    # k-strips per W DMA (1 MB transfers)
MB = 1024                       # tokens per SBUF-cached x block

F32 = mybir.dt.float32
F32R = mybir.dt.float32r


def build_nc(m_per_core: int = M_PER_CORE, mb: int = MB):
    assert m_per_core % P == 0
    mb = min(mb, m_per_core)
    m_blocks = -(-m_per_core // mb)
    nc = bacc.Bacc("TRN2", target_bir_lowering=False, debug=False)

    with tile.TileContext(nc) as tc:
        with tc.tile_pool(name="dram", bufs=1, space="DRAM") as dram:
            xt = dram.tile(
                [P, KO, m_per_core], F32R, kind="ExternalInput", name="xt",
                uniquify=False,
            )
            wt = dram.tile(
                [P, KO, OUT_F], F32R, kind="ExternalInput", name="wt", uniquify=False
            )
            bias_in = dram.tile(
                [P, OUT_F], F32, kind="ExternalInput", name="bias", uniquify=False
            )
            out_d = dram.tile(
                [P, m_per_core // P, OUT_F], F32, kind="ExternalOutput", name="out",
                uniquify=False,
            )

        n_chunks = KO // KO_CHUNK
        with (
            tc.tile_pool(name="const", bufs=1) as const,
            tc.tile_pool(name="xpool", bufs=n_chunks + 1) as xpool,
            tc.tile_pool(name="wpool", bufs=3) as wpool,
            tc.tile_pool(name="opool", bufs=4) as opool,
            tc.tile_pool(name="pspool", bufs=8, space="PSUM") as pspool,
        ):
            bias_sb = const.tile([P, OUT_F], F32, name="bias_sb")
            nc.gpsimd.dma_start(bias_sb[:], bias_in[:])

            for blk in range(m_blocks):
                cur_mb = min(mb, m_per_core - blk * mb)
                mt = cur_mb // P
                # x block cached as independent per-k-chunk tiles so the next
                # block's loads pipeline behind this block's last n-tile.
                xchunks = []
                for ko4 in range(n_chunks):
                    ks = slice(ko4 * KO_CHUNK, (ko4 + 1) * KO_CHUNK)
                    xc = xpool.tile([P, KO_CHUNK, cur_mb], F32R, name="xc")
                    nc.gpsimd.dma_start(
                        xc[:], xt[:, ks, blk * mb : blk * mb + cur_mb]
                    )
                    xchunks.append(xc)

                for nt in range(NT):
                    ns = slice(nt * NTILE, (nt + 1) * NTILE)
                    psums = [
                        pspool.tile([P, NTILE], F32, name="ps") for _ in range(mt)
                    ]
                    for ko4 in range(n_chunks):
                        ks = slice(ko4 * KO_CHUNK, (ko4 + 1) * KO_CHUNK)
                        wk = wpool.tile([P, KO_CHUNK, NTILE], F32R, name="wk")
                        nc.sync.dma_start(wk[:], wt[:, ks, ns])
                        for kj in range(KO_CHUNK):
                            ko = ko4 * KO_CHUNK + kj
                            for m in range(mt):
                                nc.tensor.matmul(
                                    psums[m][:],
                                    lhsT=xchunks[ko4][
                                        :, kj : kj + 1, m * P : (m + 1) * P
                                    ],
                                    rhs=wk[:, kj : kj + 1, :],
                                    start=(ko == 0),
                                    stop=(ko == KO - 1),
                                )
                    for m in range(mt):
                        ot = opool.tile([P, NTILE], F32, name="ot")
                        nc.vector.tensor_add(
                            out=ot[:], in0=psums[m][:], in1=bias_sb[:, ns]
                        )
                        nc.scalar.dma_start(out_d[:, blk * (mb // P) + m, ns], ot[:])

    nc.compile()
    return nc


def prep_inputs(x, W, b, A, m_per_core: int = M_PER_CORE, n_cores: int = N_CORES):
    """Host-side shard + layout prep. Returns in_maps for run_bass_kernel_spmd."""
    x = np.asarray(x, dtype=np.float32)
    W = np.asarray(W, dtype=np.float32)
    b = np.asarray(b, dtype=np.float32)
    A = np.asarray(A, dtype=np.float32)

    r = A.shape[0]
    w_eff = W + np.tile(A, (OUT_F // r, IN_F // r))
    # wt[p, ko, o] = w_eff[o, ko*128 + p]
    wt = np.ascontiguousarray(w_eff.reshape(OUT_F, KO, P).transpose(2, 1, 0))
    bias = np.ascontiguousarray(np.broadcast_to(b, (P, OUT_F)))

    x_flat = x.reshape(TOKENS, IN_F)
    in_maps = []
    for c in range(n_cores):
        shard = x_flat[c * m_per_core : (c + 1) * m_per_core]
        # xt[p, ko, m] = shard[m, ko*128 + p]
        xt = np.ascontiguousarray(shard.reshape(m_per_core, KO, P).transpose(2, 1, 0))
        in_maps.append({"xt": xt, "wt": wt, "bias": bias})
    return in_maps


def unshard(results, m_per_core: int = M_PER_CORE):
    shards = []
    for res in results:
        o = res["out"]  # [P, m_per_core//P, OUT_F]; token = strip*128 + p
        shards.append(o.transpose(1, 0, 2).reshape(m_per_core, OUT_F))
    return np.concatenate(shards, axis=0).reshape(B, S, OUT_F)


_NC_CACHE = {}


def run(x, W, b, A, trace=False, tmpdir=None, **spmd_kwargs):
    key = (M_PER_CORE, MB)
    if key not in _NC_CACHE:
        _NC_CACHE[key] = build_nc()
    nc = _NC_CACHE[key]
    in_maps = prep_inputs(x, W, b, A)
    br = run_bass_kernel_spmd(
        nc, in_maps, list(range(N_CORES)), trace=trace, tmpdir=tmpdir, **spmd_kwargs
    )
    return unshard(br.results), br


def kernel(x, W, b, A):
    out, _ = run(x, W, b, A)
    return out.astype(np.float32)
